# revision 5
# baseline (speedup 1.0000x reference)
"""Canny edge detection (nn_CannyEdge_83330955477492) on 8 Trainium2 cores.

Pipeline reproduced from the reference:
  - The reference's "gaussian blur" (sigma=0.05, and a 2x2 kernel thanks to
    arange(-(3//2)+1, 3//2+1) == [0,1]) is exactly a top-left crop of the
    reflect-padded image: blur[i,j] = x[R(i-1), R(j-1)], R(-1)=1 -> 1025x1025.
  - Sobel gx/gy on the reflect-padded blur (correlation).
  - Direction binning via exact slope comparisons on the squared gradients
    (T^2*gx^2 vs gy^2 == T*|gx| vs |gy|), magnitudes as gx^2+gy^2, NMS via
    shifted maxes of per-bin masked magnitudes, thresholds at 50^2/100^2.

Sharding: pure data parallel, 2 images per core.

Performance layout (vs the original 598us version):
  - Inputs are pre-tiled on the host into one contiguous (21 x WA) block per
    (chunk, partition), so every input DMA is 122 descriptors of ~9.9KB
    instead of 2562 descriptors of 472B (DMA-engine time ~6x lower).
  - The three outputs are fused into one f16 DRAM tensor (values 0/255/255.5
    are exact in f16) written with one DMA per chunk, packed contiguously per
    partition; the host unpacks/casts.
  - Element-wise work is split across DVE, Pool (gpsimd) and Act engines
    (the original ran ~94% of it on DVE): Pool takes the fused
    scalar_tensor_tensor ops and most NMS maxes (0.6 Q7 efficiency => 1.39
    ns/elem) while DVE keeps plain tensor_tensor ops (1.04 ns/elem), Act the
    squares and output scalings.
"""
import numpy as np

# ---------------------------------------------------------------- geometry
NIMG = 2            # images per core
H = 1024            # input image size
HO = 1025           # output size (blur is 1025x1025)
RPP = 17            # output rows per partition
PPI = 61            # partitions per image (61*17 = 1037 >= 1025)
NPART = NIMG * PPI  # 122
QROWS_PER_IMG = RPP * PPI        # 1037 (input Q image stride, rows)
QROWS = NIMG * QROWS_PER_IMG + 4  # 2078 (tail pad for last partition's window)
QCOLS = 1032        # 1 (zero "P col -1") + 1027 P cols + 4 pad
OROWS_PER_IMG = RPP * PPI        # 1037
OROWS = NIMG * OROWS_PER_IMG     # 2074
OCOLS = HO          # 1025

# column chunking: widths summing to 1025
CHUNKS = [114] * 8 + [113]
assert sum(CHUNKS) == HO
NCHUNK = len(CHUNKS)
CWMAX = max(CHUNKS)
WAMAX = CWMAX + 4
WBMAX = CWMAX + 2

_T1S = float(np.float32(np.float32(np.tan(np.deg2rad(22.5))) ** 2))
_T2S = float(np.float32(np.float32(np.tan(np.deg2rad(67.5))) ** 2))
MIN2 = float(np.float32(50.0 * 50.0))
MAX2 = float(np.float32(100.0 * 100.0))

_NC = None
LAST_RESULTS = None  # stashed BassKernelResults for test.py


# ------------------------------------------------- walrus 1-wait workaround
def _split_multiwaits(nc):
    """This walrus build rejects >1 sync wait per instruction: move extra
    waits onto fresh same-engine NOPs inserted right before the carrier."""
    import concourse.mybir as mybir

    n_split = 0
    for fn in nc.m.functions:
        for bb in fn.blocks:
            insts = list(bb.instructions)
            if not any(
                i.sync_info is not None
                and i.sync_info.on_wait
                and len(i.sync_info.on_wait) > 1
                for i in insts
            ):
                continue
            out = []
            for inst in insts:
                si = inst.sync_info
                if si is not None and si.on_wait and len(si.on_wait) > 1:
                    waits = list(si.on_wait)
                    eng = nc.engines[inst.engine]
                    for w in waits[:-1]:
                        nop = eng.nop(hint="waitsplit")
                        # eng.nop() appended to nc.cur_bb — remove it there
                        # (it must live ONLY at its split position, else the
                        # duplicate runs after sem cleanup and deadlocks).
                        host = nc.cur_bb.bb
                        lst = list(host.instructions)
                        assert lst and lst[-1].name == nop.ins.name
                        _set_insts(host, lst[:-1])
                        nop.ins.sync_info = mybir.SyncInfo(
                            on_wait=[w], on_update=[]
                        )
                        out.append(nop.ins)
                        n_split += 1
                    si.on_wait = waits[-1:]
                out.append(inst)
            _set_insts(bb, out)
    return n_split


def _set_insts(bb, lst):
    try:
        bb.instructions = lst
    except Exception:
        bb.instructions.clear()
        bb.instructions.extend(lst)


def _flat(t):
    """[P, a, b] tile -> [P, a*b] AP (free dims are contiguous in SBUF)."""
    return t[:].rearrange("p a b -> p (a b)")


# ------------------------------------------------------------ device build
def _build_nc():
    import concourse.bass as bass
    import concourse.tile as tile
    import concourse.mybir as mybir
    from concourse.ap import AP

    f32 = mybir.dt.float32
    f16 = mybir.dt.float16
    Alu = mybir.AluOpType
    Act = mybir.ActivationFunctionType

    nc = bass.Bass("TRN2", target_bir_lowering=False, debug=False, num_devices=8)
    # host-pre-tiled input: one contiguous (21, WA) block per (chunk, part)
    qp = nc.declare_dram_parameter("qp", [NCHUNK, NPART, 21 * WAMAX], f32,
                                   isOutput=False)
    bmask = nc.declare_dram_parameter("bmask", [NPART, 19 * WBMAX], f16,
                                      isOutput=False)
    # fused packed output: [chunk][part][img|week|sure][17][cw] (f16 exact)
    o_all = nc.declare_dram_parameter("o_all", [NCHUNK, NPART, 3 * RPP * CWMAX],
                                      f16, isOutput=True)

    with tile.TileContext(nc) as tc:
        with (
            tc.tile_pool(name="io2", bufs=2) as io2,    # load/store overlap
            tc.tile_pool(name="mid", bufs=1) as mid,    # per-chunk intermediates
            tc.tile_pool(name="cst", bufs=1) as cst,    # persistent constants
        ):
            # border-row mask: zeros at ang rows outside the image
            # (compute APs can't start mid-quadrant, so memsets on partitions
            # 60/61/121 are rejected by the verifier -> mask multiply instead)
            bm = cst.tile([NPART, 19, WBMAX], f16, tag="bm")
            nc.sync.dma_start(out=_flat(bm), in_=bmask[:])
            for ci, cw in enumerate(CHUNKS):
                first = ci == 0
                last = ci == NCHUNK - 1
                WA = cw + 4   # loaded cols
                WB = cw + 2   # ang cols
                # ---- load: 122 contiguous descriptors from the pre-tiled qp
                tin = io2.tile([NPART, 21, WAMAX], f32, tag="tin")
                nc.sync.dma_start(out=_flat(tin), in_=qp[ci])
                # ---- horizontal stencils (21 rows, WB cols)
                tt = mid.tile([NPART, 21, WBMAX], f32, tag="tt")
                nc.gpsimd.tensor_tensor(
                    out=tt[:, :, 0:WB], in0=tin[:, :, 0:WB], in1=tin[:, :, 2:WA],
                    op=Alu.add)
                rsm = mid.tile([NPART, 21, WBMAX], f32, tag="rsm")
                nc.vector.scalar_tensor_tensor(
                    out=rsm[:, :, 0:WB], in0=tin[:, :, 1:WB + 1], scalar=2.0,
                    in1=tt[:, :, 0:WB], op0=Alu.mult, op1=Alu.add)
                dd = mid.tile([NPART, 21, WBMAX], f32, tag="dd")
                nc.gpsimd.tensor_tensor(
                    out=dd[:, :, 0:WB], in0=tin[:, :, 2:WA], in1=tin[:, :, 0:WB],
                    op=Alu.subtract)

                # ---- vertical stencils (19 rows): gx, gy
                t2 = mid.tile([NPART, 19, WBMAX], f32, tag="t2")
                nc.gpsimd.tensor_tensor(
                    out=t2[:, :, 0:WB], in0=dd[:, 0:19, 0:WB],
                    in1=dd[:, 2:21, 0:WB], op=Alu.add)
                gx = mid.tile([NPART, 19, WBMAX], f32, tag="gx")
                nc.vector.scalar_tensor_tensor(
                    out=gx[:, :, 0:WB], in0=dd[:, 1:20, 0:WB], scalar=2.0,
                    in1=t2[:, :, 0:WB], op0=Alu.mult, op1=Alu.add)
                gy = mid.tile([NPART, 19, WBMAX], f32, tag="gy")
                nc.gpsimd.tensor_tensor(
                    out=gy[:, :, 0:WB], in0=rsm[:, 2:21, 0:WB],
                    in1=rsm[:, 0:19, 0:WB], op=Alu.subtract)

                # ---- sg, squares (ACT, bit-exact), bin predicates from
                # squares (T^2 gx^2 >= gy^2 <=> T|gx| >= |gy|), magnitude^2
                sg = mid.tile([NPART, 19, WBMAX], f32, tag="t2")  # t2 dead
                nc.gpsimd.tensor_tensor(out=sg[:, :, 0:WB], in0=gx[:, :, 0:WB],
                                        in1=gy[:, :, 0:WB], op=Alu.mult)
                gx2 = mid.tile([NPART, 19, WBMAX], f32, tag="tt")  # tt dead
                nc.scalar.activation(out=gx2[:, :, 0:WB], in_=gx[:, :, 0:WB],
                                     func=Act.Square)
                gy2 = mid.tile([NPART, 19, WBMAX], f32, tag="dd")  # dd dead
                nc.scalar.activation(out=gy2[:, :, 0:WB], in_=gy[:, :, 0:WB],
                                     func=Act.Square)
                c0 = mid.tile([NPART, 19, WBMAX], f32, tag="c0")
                nc.vector.scalar_tensor_tensor(
                    out=c0[:, :, 0:WB], in0=gx2[:, :, 0:WB], scalar=_T1S,
                    in1=gy2[:, :, 0:WB], op0=Alu.mult, op1=Alu.is_ge)
                d2 = mid.tile([NPART, 19, WBMAX], f32, tag="d2")
                nc.vector.scalar_tensor_tensor(
                    out=d2[:, :, 0:WB], in0=gx2[:, :, 0:WB], scalar=_T2S,
                    in1=gy2[:, :, 0:WB], op0=Alu.mult, op1=Alu.is_gt)
                mm = mid.tile([NPART, 19, WBMAX], f32, tag="mm")
                nc.gpsimd.tensor_tensor(out=mm[:, :, 0:WB], in0=gx2[:, :, 0:WB],
                                        in1=gy2[:, :, 0:WB], op=Alu.add)

                # ---- zero magnitude outside the image (NMS zero-padding)
                nc.gpsimd.tensor_tensor(out=mm[:, :, 0:WB], in0=mm[:, :, 0:WB],
                                        in1=bm[:, :, 0:WB], op=Alu.mult)
                if first:
                    nc.gpsimd.memset(mm[:, :, 0:1], 0.0)      # ang col -1
                if last:
                    nc.gpsimd.memset(mm[:, :, WB - 1:WB], 0.0)  # ang col 1025

                # ---- per-bin masked magnitudes + NMS
                # bins are disjoint and ang_b == M at in-bin pixels, so
                # P100_b == P50_b * [M >= 100^2]: one center-extent compare
                # replaces the whole per-bin MAX2 pass.
                qts = [mid.tile([NPART, RPP, CWMAX], f32, tag=f"qt{i}",
                                name=f"qt{i}") for i in range(2)]
                cmps = [mid.tile([NPART, RPP, CWMAX], f16, tag=f"cmp{i}",
                                 name=f"cmp{i}") for i in range(4)]

                def nms_bin(bno, ang, r1, c1, r2, c2, qt_eng):
                    qt = qts[bno % 2]
                    qt_eng.tensor_tensor(
                        out=qt[:, :, 0:cw],
                        in0=ang[:, r1:r1 + RPP, c1:c1 + cw],
                        in1=ang[:, r2:r2 + RPP, c2:c2 + cw],
                        op=Alu.max)
                    cen = ang[:, 1:18, 1:1 + cw]
                    nc.vector.scalar_tensor_tensor(
                        out=cmps[bno][:, :, 0:cw], in0=qt[:, :, 0:cw],
                        scalar=MIN2, in1=cen, op0=Alu.max, op1=Alu.is_le)

                md2 = mid.tile([NPART, 19, WBMAX], f32, tag="md2")
                nc.gpsimd.tensor_tensor(out=md2[:, :, 0:WB], in0=mm[:, :, 0:WB],
                                        in1=d2[:, :, 0:WB], op=Alu.mult)
                angA = mid.tile([NPART, 19, WBMAX], f32, tag="angA")
                nc.gpsimd.tensor_tensor(out=angA[:, :, 0:WB], in0=mm[:, :, 0:WB],
                                        in1=md2[:, :, 0:WB], op=Alu.subtract)
                nms_bin(0, angA, 0, 1, 2, 1, nc.vector)   # bin2: up/down
                angB = mid.tile([NPART, 19, WBMAX], f32, tag="angB")
                nc.vector.tensor_tensor(out=angB[:, :, 0:WB], in0=md2[:, :, 0:WB],
                                        in1=c0[:, :, 0:WB], op=Alu.mult)
                nms_bin(1, angB, 1, 0, 1, 2, nc.vector)   # bin0: left/right
                # mdiag = Md2 - ang0 (in place on md2)
                nc.vector.tensor_tensor(out=md2[:, :, 0:WB], in0=md2[:, :, 0:WB],
                                        in1=angB[:, :, 0:WB], op=Alu.subtract)
                angC = mid.tile([NPART, 19, WBMAX], f32, tag="angB")
                nc.vector.scalar_tensor_tensor(
                    out=angC[:, :, 0:WB], in0=sg[:, :, 0:WB], scalar=0.0,
                    in1=md2[:, :, 0:WB], op0=Alu.is_gt, op1=Alu.mult)
                nms_bin(2, angC, 0, 0, 2, 2, nc.vector)   # bin3: main diag
                # ang1 = mdiag - ang3 (in place on md2)
                nc.vector.tensor_tensor(out=md2[:, :, 0:WB], in0=md2[:, :, 0:WB],
                                        in1=angC[:, :, 0:WB], op=Alu.subtract)
                nms_bin(3, md2, 0, 2, 2, 0, nc.vector)    # bin1: anti diag

                # ---- accumulate passes (tree, in place on the cmp tiles),
                # split weak/sure, scale out
                nc.vector.tensor_tensor(
                    out=_fl(cmps[0], cw), in0=_fl(cmps[0], cw),
                    in1=_fl(cmps[1], cw), op=Alu.add)
                nc.vector.tensor_tensor(
                    out=_fl(cmps[2], cw), in0=_fl(cmps[2], cw),
                    in1=_fl(cmps[3], cw), op=Alu.add)
                e50 = cmps[1]
                nc.vector.tensor_tensor(
                    out=_fl(e50, cw), in0=_fl(cmps[0], cw), in1=_fl(cmps[2], cw),
                    op=Alu.add)
                big = mid.tile([NPART, RPP, CWMAX], f16, tag="big")
                nc.vector.tensor_scalar(
                    out=big[:, :, 0:cw], in0=mm[:, 1:18, 1:1 + cw],
                    scalar1=MAX2, scalar2=None, op0=Alu.is_ge)
                sure = cmps[3]
                nc.vector.tensor_tensor(
                    out=_fl(sure, cw), in0=_fl(e50, cw), in1=_fl(big, cw),
                    op=Alu.mult)
                week = big
                nc.vector.tensor_tensor(
                    out=_fl(week, cw), in0=_fl(e50, cw), in1=_fl(sure, cw),
                    op=Alu.subtract)

                # scale to output values on ACT (exact: inputs are 0/1)
                eout = io2.tile([NPART, 3, RPP, CWMAX], f16, tag="eout")
                nc.scalar.activation(out=eout[:, 0, :, 0:cw], in_=e50[:, :, 0:cw],
                                     func=Act.Copy, scale=255.5)
                nc.scalar.activation(out=eout[:, 1, :, 0:cw], in_=week[:, :, 0:cw],
                                     func=Act.Copy, scale=255.0)
                nc.scalar.activation(out=eout[:, 2, :, 0:cw], in_=sure[:, :, 0:cw],
                                     func=Act.Copy, scale=255.0)
                nc.sync.dma_start(
                    out=o_all[ci],
                    in_=eout[:].rearrange("p a b c -> p (a b c)"))

    _split_multiwaits(nc)
    return nc


def _fl(t, cw):
    """[P, RPP, CWMAX] tile -> flat [P, RPP*cw]-ish AP. When cw == CWMAX the
    free dims are contiguous (enables DVE 2x f16 mode); otherwise keep the
    2D strided view."""
    if cw == CWMAX:
        return t[:].rearrange("p a b -> p (a b)")
    return t[:, :, 0:cw]


def _get_nc():
    global _NC
    if _NC is None:
        _NC = _build_nc()
    return _NC


# ------------------------------------------------------------- host helpers
def _build_qp(images):
    """images: (16, 1024, 1024) f32 -> per-core pre-tiled QP
    (8, NCHUNK, NPART, 21*WAMAX).

    Q[img_block] row r, col c = P[r-1, c-1] where P is the twice-padded
    image: P index list (both dims) = [0, 1, 0, 1, 2, ..., 1023, 1022].
    QP[core, ci, p, j*WAMAX + k] = Q[17p + j, a_ci + k]."""
    idx = np.empty(1027, np.int64)
    idx[0] = 0
    idx[1] = 1
    idx[2:1026] = np.arange(1024)
    idx[1026] = 1022
    rowidx = (17 * np.arange(NPART)[:, None] + np.arange(21)[None, :])  # [122,21]
    offs = np.concatenate([[0], np.cumsum(CHUNKS)[:-1]])
    qps = np.empty((8, NCHUNK, NPART, 21 * WAMAX), np.float32)
    for core in range(8):
        q = np.zeros((QROWS, QCOLS), np.float32)
        for k in range(NIMG):
            im = images[core * NIMG + k]
            p = im[idx][:, idx]  # (1027, 1027)
            base = k * QROWS_PER_IMG
            q[base + 1: base + 1028, 1:1028] = p
        qr = q[rowidx]  # [122, 21, QCOLS]
        for ci, (a, cwc) in enumerate(zip(offs, CHUNKS)):
            blk = np.zeros((NPART, 21, WAMAX), np.float32)
            blk[:, :, 0:cwc + 4] = qr[:, :, a:a + cwc + 4]
            qps[core, ci] = blk.reshape(NPART, -1)
    return qps


def kernel(images):
    global LAST_RESULTS
    from concourse.bass_utils import run_bass_kernel_spmd

    images = np.asarray(images, dtype=np.float32)
    assert images.shape == (16, 1024, 1024, 1), images.shape
    qps = _build_qp(images[:, :, :, 0])

    bm = np.ones((NPART, 19, WBMAX), np.float16)
    for base in (0, PPI):
        bm[base, 0, :] = 0.0          # ang row -1 of each image
        bm[base + PPI - 1, 6:, :] = 0.0  # ang rows >= 1025 of each image
    bm = bm.reshape(NPART, -1)

    nc = _get_nc()
    in_maps = [{"qp": qps[c], "bmask": bm} for c in range(8)]
    res = run_bass_kernel_spmd(nc, in_maps, list(range(8)))
    LAST_RESULTS = res

    offs = np.concatenate([[0], np.cumsum(CHUNKS)[:-1]])
    out = []
    full3 = np.empty((8, 3, OROWS, OCOLS), np.float32)
    for c in range(8):
        r = res.results[c]["o_all"].reshape(NCHUNK, NPART, 3, RPP, CWMAX)
        for ci, (a, cwc) in enumerate(zip(offs, CHUNKS)):
            blk = r[ci, :, :, :, 0:cwc].astype(np.float32)  # [122,3,17,cw]
            full3[c, :, :, a:a + cwc] = (
                blk.transpose(1, 0, 2, 3).reshape(3, OROWS, cwc))
    for j in range(3):
        full = np.empty((16, HO, HO, 1), np.float32)
        for c in range(8):
            r = full3[c, j].reshape(NIMG, OROWS_PER_IMG, OCOLS)
            full[c * NIMG: c * NIMG + NIMG, :, :, 0] = r[:, :HO, :]
        out.append(full)
    return tuple(out)
